# revision 10
# baseline (speedup 1.0000x reference)
"""Trainium2 Bass kernel: 3-layer heterogeneous graph attention encoder.

Sharding: dst-node (edge-cut) partition over 8 NeuronCores. Each core owns a
contiguous range of furniture and room nodes (padded to multiples of 128),
computes the dense per-node projections for its rows, all-gathers the
projection tables, then processes the edges whose destination it owns:
random-row dma_gather of source projections, per-edge softmax weights via
fused one-hot ops, and PSUM-accumulated one-hot matmul aggregation per
128-dst window.
"""

import math
import numpy as np

import concourse.bass as bass
import concourse.bacc as bacc
import concourse.tile as tile
from concourse import mybir
from concourse import bass_utils
from concourse.masks import make_identity

P = 128
NCORES = 8
FURN_DIMS = [(1040, 256), (256, 128), (128, 64)]
ROOM_D = 14
LEAKY = 0.01
NEGBIG = -1.0e30
EPS = 1.0e-9


class Cfg:
    def __init__(self, n_furn, n_room):
        assert n_furn % NCORES == 0 and n_room % NCORES == 0
        self.NF, self.NR = n_furn, n_room
        self.nfc_real = n_furn // NCORES
        self.nrc_real = n_room // NCORES
        self.NFC = ((self.nfc_real + P - 1) // P) * P
        self.NRC = ((self.nrc_real + P - 1) // P) * P
        self.FWIN = self.NFC // P
        self.RWIN = self.NRC // P
        self.NF_PAD = self.NFC * NCORES
        self.NR_PAD = self.NRC * NCORES
        # A/B split for int16 gather indices (furniture tables only)
        self.SPLIT = (self.NF_PAD // 2 + P - 1) // P * P
        assert self.SPLIT < 32768 and self.NF_PAD - self.SPLIT <= 32768
        assert self.NR_PAD < 32768
        # table row strides (fp32 elems, multiples of 64) and rr part offset
        self.SF = []
        self.SR = []
        self.RRO = []
        for (_, d) in FURN_DIMS:
            self.SF.append(((d + 2 + 63) // 64) * 64)
            rro = ((d + 2 + 63) // 64) * 64
            self.RRO.append(rro)
            self.SR.append(rro + 64)

    def furn_pad_id(self, orig):
        return orig + (self.NFC - self.nfc_real) * (orig // self.nfc_real)

    def room_pad_id(self, orig):
        return orig + (self.NRC - self.nrc_real) * (orig // self.nrc_real)


def _prep_rel(cfg, src_new, dst_new, efeat, n_core_pad, nwin, split=None):
    """Per-core padded edge-slot arrays for one relation.

    Returns (T_list, per_core) where per_core[c] is a dict of numpy arrays and
    T_list gives uniform tiles-per-window for each src group.
    """
    c = efeat.shape[1]
    ngrp = 2 if split is not None else 1
    owner = dst_new // n_core_pad
    ldst = dst_new % n_core_pad
    win = ldst // P
    dloc = (ldst % P).astype(np.float32)
    grp = (src_new >= split).astype(np.int64) if split is not None else np.zeros_like(src_new)

    key = (owner * nwin + win) * ngrp + grp
    order = np.argsort(key, kind="stable")
    key_s = key[order]
    src_s = src_new[order]
    dloc_s = dloc[order]
    ef_s = efeat[order]

    nbuck = NCORES * nwin * ngrp
    starts = np.searchsorted(key_s, np.arange(nbuck))
    ends = np.searchsorted(key_s, np.arange(nbuck) + 1)
    counts = (ends - starts).reshape(NCORES, nwin, ngrp)
    T = [int(math.ceil(max(1, counts[:, :, g].max()) / P)) for g in range(ngrp)]
    sumT = sum(T)
    toff = np.concatenate([[0], np.cumsum(T)])[:-1]

    per_core = []
    for co in range(NCORES):
        nslot = nwin * sumT * P
        idx_sl = np.zeros(nslot, np.int64)
        dl_sl = np.zeros(nslot, np.float32)
        pb_sl = np.full(nslot, NEGBIG, np.float32)
        ef_sl = np.zeros((nslot, c), np.float32)
        for w in range(nwin):
            for g in range(ngrp):
                b = (co * nwin + w) * ngrp + g
                s, e = starts[b], ends[b]
                n = e - s
                base = (w * sumT + toff[g]) * P
                if n:
                    sl = slice(base, base + n)
                    ids = src_s[s:e]
                    idx_sl[sl] = ids - (split if g == 1 else 0)
                    dl_sl[sl] = dloc_s[s:e]
                    pb_sl[sl] = 0.0
                    ef_sl[sl] = ef_s[s:e]
        ntiles = nslot // P
        dl_arr = np.ascontiguousarray(dl_sl.reshape(ntiles, P).T)
        pb_arr = np.ascontiguousarray(pb_sl.reshape(ntiles, P).T)
        ef_arr = np.ascontiguousarray(
            ef_sl.reshape(ntiles, P, c).transpose(1, 0, 2).reshape(P, ntiles * c))
        # idx array: per (window, group) call stream chunked to <=1024 idxs
        blocks = []
        for w in range(nwin):
            for g in range(ngrp):
                base = (w * sumT + toff[g]) * P
                callsl = idx_sl[base:base + T[g] * P]
                for off in range(0, T[g] * P, 1024):
                    ni = min(1024, T[g] * P - off)
                    blk = callsl[off:off + ni].reshape(ni // 16, 16).T.astype(np.int16)
                    blocks.append(np.tile(blk, (8, 1)))
        idx_arr = np.ascontiguousarray(np.concatenate(blocks, axis=1))
        per_core.append(dict(idx=idx_arr, dl=dl_arr, pb=pb_arr, ef=ef_arr))
    return T, per_core


def _host_prep(cfg, inputs):
    """All host-side graph/weight prep. Returns (in_maps, T-dict, smalls)."""
    p = inputs["params"]
    ffs = cfg.furn_pad_id(np.asarray(inputs["ff_src"], np.int64))
    ffd = cfg.furn_pad_id(np.asarray(inputs["ff_dst"], np.int64))
    rrs = cfg.room_pad_id(np.asarray(inputs["rr_src"], np.int64))
    rrd = cfg.room_pad_id(np.asarray(inputs["rr_dst"], np.int64))
    rfs = cfg.room_pad_id(np.asarray(inputs["rf_src"], np.int64))
    rfd = cfg.furn_pad_id(np.asarray(inputs["rf_dst"], np.int64))

    T_ff, pc_ff = _prep_rel(cfg, ffs, ffd, np.asarray(inputs["e_ff"], np.float32),
                            cfg.NFC, cfg.FWIN, split=cfg.SPLIT)
    T_rr, pc_rr = _prep_rel(cfg, rrs, rrd, np.asarray(inputs["e_rr"], np.float32),
                            cfg.NRC, cfg.RWIN)
    T_rf, pc_rf = _prep_rel(cfg, rfs, rfd, np.asarray(inputs["e_rf"], np.float32),
                            cfg.NFC, cfg.FWIN)

    # node features: per-core feature-major shards, padded
    xf = np.asarray(inputs["x_furn"], np.float32)
    xr = np.asarray(inputs["x_room"], np.float32)
    xftl, xrtl = [], []
    for co in range(NCORES):
        sh = np.zeros((cfg.NFC, xf.shape[1]), np.float32)
        sh[:cfg.nfc_real] = xf[co * cfg.nfc_real:(co + 1) * cfg.nfc_real]
        xftl.append(np.ascontiguousarray(sh.T))
        shr = np.zeros((cfg.NRC, ROOM_D), np.float32)
        shr[:cfg.nrc_real] = xr[co * cfg.nrc_real:(co + 1) * cfg.nrc_real]
        xrtl.append(np.ascontiguousarray(shr.T))

    # big weights per layer
    wf, wr = [], []
    for l, lp in enumerate(p["layers"]):
        d = FURN_DIMS[l][1]
        ws_ff = np.asarray(lp["ff"]["Ws"], np.float32)
        cols = [ws_ff,
                (ws_ff @ np.asarray(lp["ff"]["a_s"], np.float32))[:, None],
                (np.asarray(lp["ff"]["Wd"], np.float32) @ np.asarray(lp["ff"]["a_d"], np.float32))[:, None],
                (np.asarray(lp["rf"]["Wd"], np.float32) @ np.asarray(lp["rf"]["a_d"], np.float32))[:, None]]
        wf.append(np.ascontiguousarray(np.concatenate(cols, 1)))  # [fin, d+3]
        ws_rf = np.asarray(lp["rf"]["Ws"], np.float32)
        ws_rr = np.asarray(lp["rr"]["Ws"], np.float32)
        colsr = [ws_rf,
                 (ws_rf @ np.asarray(lp["rf"]["a_s"], np.float32))[:, None],
                 ws_rr,
                 (ws_rr @ np.asarray(lp["rr"]["a_s"], np.float32))[:, None],
                 (np.asarray(lp["rr"]["Wd"], np.float32) @ np.asarray(lp["rr"]["a_d"], np.float32))[:, None]]
        wr.append(np.ascontiguousarray(np.concatenate(colsr, 1)))  # [14, d+17]

    whm = np.ascontiguousarray(np.asarray(p["wMean"]["W"], np.float32))
    whv = np.ascontiguousarray(np.asarray(p["wLogVar"]["W"], np.float32))
    bm = np.zeros((P, 1), np.float32)
    bm[:64, 0] = np.asarray(p["wMean"]["b"], np.float32)
    bv = np.zeros((P, 1), np.float32)
    bv[:64, 0] = np.asarray(p["wLogVar"]["b"], np.float32)

    iota = np.ascontiguousarray(
        np.arange(P, dtype=np.float32)[None, :].repeat(P, 0))

    smalls = dict(
        a_e=[[np.asarray(lp[r]["a_e"], np.float32) for r in ("ff", "rr", "rf")]
             for lp in p["layers"]],
        We=[[np.asarray(lp[r]["We"], np.float32) for r in ("ff", "rr", "rf")]
            for lp in p["layers"]],
        be=[[np.asarray(lp[r]["be"], np.float32) for r in ("ff", "rr", "rf")]
            for lp in p["layers"]],
    )

    in_maps = []
    for co in range(NCORES):
        m = dict(
            xft=xftl[co], xrt=xrtl[co],
            whm=whm, whv=whv, bm=bm, bv=bv, iota=iota,
            idx_ff=pc_ff[co]["idx"], dl_ff=pc_ff[co]["dl"],
            pb_ff=pc_ff[co]["pb"], e_ff=pc_ff[co]["ef"],
            idx_rr=pc_rr[co]["idx"], dl_rr=pc_rr[co]["dl"],
            pb_rr=pc_rr[co]["pb"], e_rr=pc_rr[co]["ef"],
            idx_rf=pc_rf[co]["idx"], dl_rf=pc_rf[co]["dl"],
            pb_rf=pc_rf[co]["pb"], e_rf=pc_rf[co]["ef"],
        )
        for l in range(3):
            m[f"wf{l}"] = wf[l]
            m[f"wr{l}"] = wr[l]
        in_maps.append(m)
    return in_maps, dict(ff=T_ff, rr=T_rr, rf=T_rf), smalls


def _gather_calls(T):
    """(tile_offset, ntiles) chunks per group call stream, <=8 tiles each."""
    out = []
    for g, t in enumerate(T):
        chunks = []
        off = 0
        while off < t:
            n = min(8, t - off)
            chunks.append((off, n))
            off += n
        out.append(chunks)
    return out


def _build(cfg, T, smalls, in0_shapes):
    f32 = mybir.dt.float32
    i16 = mybir.dt.int16
    AF = mybir.ActivationFunctionType
    OP = mybir.AluOpType
    nc = bacc.Bacc("TRN2", target_bir_lowering=False, num_devices=NCORES,
                   num_swdge_queues=4)
    qctr = [0]

    def nextq():
        qctr[0] = (qctr[0] + 1) % 4
        return qctr[0]

    # ---------------- I/O -----------------
    inp = {}
    for name, shp in in0_shapes.items():
        dt = i16 if name.startswith("idx") else f32
        inp[name] = nc.dram_tensor(name, list(shp), dt, kind="ExternalInput")
    mu_out = nc.dram_tensor("mu", [cfg.NFC, 64], f32, kind="ExternalOutput")
    lv_out = nc.dram_tensor("lv", [cfg.NFC, 64], f32, kind="ExternalOutput")

    relT = {"ff": T["ff"], "rr": T["rr"], "rf": T["rf"]}
    sumT_ff = sum(relT["ff"])
    sumT_rr = sum(relT["rr"])
    sumT_rf = sum(relT["rf"])
    rg = [list(range(NCORES))]

    from contextlib import ExitStack
    _stk = ExitStack()
    with tile.TileContext(nc) as tc:
        dram = _stk.enter_context(tc.tile_pool(name="dram", bufs=1, space="DRAM"))
        const = _stk.enter_context(tc.tile_pool(name="const", bufs=1))

        # DRAM scratch
        ftab_sh = [dram.tile([cfg.NFC, cfg.SF[l]], f32, tag=f"fts{l}", name=f"fts{l}") for l in range(3)]
        ftab = [dram.tile([cfg.NF_PAD, cfg.SF[l]], f32, tag=f"ftf{l}", name=f"ftf{l}", addr_space="Shared") for l in range(3)]
        rtab_sh = [dram.tile([cfg.NRC, cfg.SR[l]], f32, tag=f"rts{l}", name=f"rts{l}") for l in range(3)]
        rtab = [dram.tile([cfg.NR_PAD, cfg.SR[l]], f32, tag=f"rtf{l}", name=f"rtf{l}", addr_space="Shared") for l in range(3)]
        sd_ff = [dram.tile([cfg.NFC, 1], f32, tag=f"sdf{l}", name=f"sdf{l}") for l in range(3)]
        sd_rf = [dram.tile([cfg.NFC, 1], f32, tag=f"sdr{l}", name=f"sdr{l}") for l in range(3)]
        sd_rr = [dram.tile([cfg.NRC, 1], f32, tag=f"sdq{l}", name=f"sdq{l}") for l in range(3)]
        xfT = [None,
               dram.tile([FURN_DIMS[0][1], cfg.NFC], f32, tag="xf1T", name="xf1T"),
               dram.tile([FURN_DIMS[1][1], cfg.NFC], f32, tag="xf2T", name="xf2T"),
               dram.tile([FURN_DIMS[2][1], cfg.NFC], f32, tag="xf3T", name="xf3T")]
        xrT = [None, dram.tile([14, cfg.NRC], f32, tag="xr1T", name="xr1T"),
               dram.tile([14, cfg.NRC], f32, tag="xr2T", name="xr2T")]

        # constants
        iota_t = const.tile([P, P], f32)
        nc.sync.dma_start(iota_t[:], inp["iota"][:])
        ident = const.tile([P, P], f32)
        make_identity(nc, ident[:])

        # weights to SBUF
        wf_t = []   # per layer: list of [128, cols] chunks
        wr_t = []
        for l in range(3):
            fin, d = FURN_DIMS[l]
            nk = (fin + P - 1) // P
            cols = d + 3
            chunks = []
            for k in range(nk):
                kn = min(P, fin - k * P)
                t_ = const.tile([P, cols], f32, tag=f"wf{l}_{k}")
                nc.sync.dma_start(t_[:kn, :], inp[f"wf{l}"][k * P:k * P + kn, :])
                chunks.append((t_, kn))
            wf_t.append(chunks)
            t_ = const.tile([ROOM_D, d + 17], f32, tag=f"wr{l}")
            nc.sync.dma_start(t_[:], inp[f"wr{l}"][:])
            wr_t.append(t_)
        whm_t = const.tile([64, 64], f32)
        nc.sync.dma_start(whm_t[:], inp["whm"][:])
        whv_t = const.tile([64, 64], f32)
        nc.sync.dma_start(whv_t[:], inp["whv"][:])

        # per-edge static arrays
        st_arr = {}
        for r, sumT_, c in (("ff", sumT_ff, 3), ("rr", sumT_rr, 4), ("rf", sumT_rf, 5)):
            nwin = cfg.FWIN if r in ("ff", "rf") else cfg.RWIN
            tt = nwin * sumT_
            idx_t = const.tile([P, inp[f"idx_{r}"].shape[1]], i16, tag=f"idx{r}")
            nc.sync.dma_start(idx_t[:], inp[f"idx_{r}"][:])
            dl_t = const.tile([P, tt], f32, tag=f"dl{r}")
            nc.sync.dma_start(dl_t[:], inp[f"dl_{r}"][:])
            st_arr[r] = dict(idx=idx_t, dl=dl_t, tt=tt, c=c, nwin=nwin)

        # ---------------- edge-feature prelude: g_pb per layer/relation ----
        gp = {}  # (l, r) -> [P, tt] tile
        with tc.tile_pool(name="effp", bufs=1) as effp:
            eff = {}
            for r in ("ff", "rr", "rf"):
                c = st_arr[r]["c"]
                tt = st_arr[r]["tt"]
                e0 = effp.tile([P, tt * c], f32, tag=f"e0{r}")
                nc.sync.dma_start(e0[:], inp[f"e_{r}"][:])
                pb = effp.tile([P, tt], f32, tag=f"pb{r}")
                nc.sync.dma_start(pb[:], inp[f"pb_{r}"][:])
                eff[r] = (e0, pb)
            ridx = {"ff": 0, "rr": 1, "rf": 2}
            for l in range(3):
                for r in ("ff", "rr", "rf"):
                    c = st_arr[r]["tt"], st_arr[r]["c"]
                    tt, cc = c
                    e_t, pb_t = eff[r]
                    ae = smalls["a_e"][l][ridx[r]]
                    g_t = const.tile([P, tt], f32, tag=f"gp{l}{r}")
                    ev = e_t[:].rearrange("p (t c) -> p t c", c=cc)
                    nc.vector.scalar_tensor_tensor(
                        out=g_t[:], in0=ev[:, :, 0], scalar=float(ae[0]),
                        in1=pb_t[:], op0=OP.mult, op1=OP.add)
                    for j in range(1, cc):
                        nc.vector.scalar_tensor_tensor(
                            out=g_t[:], in0=ev[:, :, j], scalar=float(ae[j]),
                            in1=g_t[:], op0=OP.mult, op1=OP.add)
                    gp[(l, r)] = g_t
                if l < 2:
                    for r in ("ff", "rr", "rf"):
                        tt, cc = st_arr[r]["tt"], st_arr[r]["c"]
                        e_t, pb_t = eff[r]
                        We = smalls["We"][l][ridx[r]]
                        be = smalls["be"][l][ridx[r]]
                        e_n = effp.tile([P, tt * cc], f32, tag=f"e{l + 1}{r}")
                        ev = e_t[:].rearrange("p (t c) -> p t c", c=cc)
                        en = e_n[:].rearrange("p (t c) -> p t c", c=cc)
                        for j2 in range(cc):
                            nc.vector.tensor_scalar(
                                out=en[:, :, j2], in0=ev[:, :, 0],
                                scalar1=float(We[0, j2]), scalar2=float(be[j2]),
                                op0=OP.mult, op1=OP.add)
                            for j in range(1, cc):
                                nc.vector.scalar_tensor_tensor(
                                    out=en[:, :, j2], in0=ev[:, :, j],
                                    scalar=float(We[j, j2]), in1=en[:, :, j2],
                                    op0=OP.mult, op1=OP.add)
                            nc.vector.tensor_scalar_max(
                                out=en[:, :, j2], in0=en[:, :, j2], scalar1=0.0)
                        eff[r] = (e_n, pb_t)

        # ---------------- per-layer phases ----------------
        for l in range(3):
            fin, D = FURN_DIMS[l]
            SF, SR, RRO = cfg.SF[l], cfg.SR[l], cfg.RRO[l]
            nk = (fin + P - 1) // P

            # ---- node phase: furniture ----
            with tc.tile_pool(name=f"nps{l}", bufs=2, space="PSUM") as nps, \
                 tc.tile_pool(name=f"nsb{l}", bufs=3) as nsb:
                for m in range(cfg.FWIN):
                    ps = nps.tile([P, D + 3], f32, tag="f")
                    for k in range(nk):
                        wt, kn = wf_t[l][k]
                        a = nsb.tile([P, P], f32, tag="xc")
                        if l == 0:
                            src = inp["xft"]
                        else:
                            src = xfT[l]
                        nc.sync.dma_start(
                            a[:kn, :], src[k * P:k * P + kn, m * P:(m + 1) * P])
                        nc.tensor.matmul(ps[:], a[:kn, :], wt[:kn, :],
                                         start=(k == 0), stop=(k == nk - 1))
                    st = nsb.tile([P, SF], f32, tag="stg")
                    nc.vector.tensor_copy(st[:, 0:D], ps[:, 0:D])
                    nc.vector.memset(st[:, D:D + 1], 1.0)
                    nc.vector.tensor_copy(st[:, D + 1:D + 4], ps[:, D:D + 3])
                    nc.sync.dma_start(ftab_sh[l][m * P:(m + 1) * P, :], st[:])
                    nc.sync.dma_start(sd_ff[l][m * P:(m + 1) * P, :], st[:, D + 2:D + 3])
                    nc.sync.dma_start(sd_rf[l][m * P:(m + 1) * P, :], st[:, D + 3:D + 4])
                # rooms
                for m in range(cfg.RWIN):
                    ps = nps.tile([P, D + 17], f32, tag="r")
                    wt = wr_t[l]
                    a = nsb.tile([ROOM_D, P], f32, tag="xr")
                    if l == 0:
                        nc.sync.dma_start(a[:], inp["xrt"][:, m * P:(m + 1) * P])
                    else:
                        nc.sync.dma_start(a[:], xrT[l][:, m * P:(m + 1) * P])
                    nc.tensor.matmul(ps[:], a[:], wt[:], start=True, stop=True)
                    st = nsb.tile([P, SR], f32, tag="stgr")
                    nc.vector.tensor_copy(st[:, 0:D], ps[:, 0:D])
                    nc.vector.memset(st[:, D:D + 1], 1.0)
                    nc.vector.tensor_copy(st[:, D + 1:D + 2], ps[:, D:D + 1])
                    nc.vector.tensor_copy(st[:, RRO:RRO + 14], ps[:, D + 1:D + 15])
                    nc.vector.memset(st[:, RRO + 14:RRO + 15], 1.0)
                    nc.vector.tensor_copy(st[:, RRO + 15:RRO + 17], ps[:, D + 15:D + 17])
                    nc.sync.dma_start(rtab_sh[l][m * P:(m + 1) * P, :], st[:])
                    nc.sync.dma_start(sd_rr[l][m * P:(m + 1) * P, :], st[:, RRO + 16:RRO + 17])

            nc.gpsimd.collective_compute(
                "AllGather", OP.bypass, ins=[ftab_sh[l].opt()],
                outs=[ftab[l].opt()], replica_groups=rg)
            nc.gpsimd.collective_compute(
                "AllGather", OP.bypass, ins=[rtab_sh[l].opt()],
                outs=[rtab[l].opt()], replica_groups=rg)

            # ---- edge phase: furniture windows (ff + rf) ----
            TA, TB = relT["ff"]
            TRF = relT["rf"][0]
            TRR = relT["rr"][0]
            with tc.tile_pool(name=f"eps{l}", bufs=2, space="PSUM") as eps, \
                 tc.tile_pool(name=f"mps{l}", bufs=2, space="PSUM") as mps, \
                 tc.tile_pool(name=f"esb{l}", bufs=2) as esb, \
                 tc.tile_pool(name=f"scr{l}", bufs=4) as scr:
                idx_ff, dl_ff = st_arr["ff"]["idx"], st_arr["ff"]["dl"]
                idx_rf, dl_rf = st_arr["rf"]["idx"], st_arr["rf"]["dl"]
                ffcalls = _gather_calls(relT["ff"])
                rfcalls = _gather_calls(relT["rf"])
                rrcalls = _gather_calls(relT["rr"])
                for w in range(cfg.FWIN):
                    # M matrices
                    Ms = {}
                    for rel, sdt in (("ff", sd_ff[l]), ("rf", sd_rf[l])):
                        sc = esb.tile([P, 1], f32, tag=f"sd{rel}")
                        nc.sync.dma_start(sc[:], sdt[w * P:(w + 1) * P, :])
                        mp = mps.tile([P, P], f32, tag="m")
                        nc.tensor.transpose(mp[:], sc[:, 0:1].to_broadcast([P, P]), ident[:])
                        mm = esb.tile([P, P], f32, tag=f"M{rel}")
                        nc.vector.tensor_copy(mm[:], mp[:])
                        Ms[rel] = mm
                    ps_ff = eps.tile([P, D + 1], f32, tag="ff")
                    ps_rf = eps.tile([P, D + 1], f32, tag="rf")

                    groups = []
                    for g in range(2):
                        groups.append(("ff", g, relT["ff"][g], ffcalls[g]))
                    groups.append(("rf", 0, TRF, rfcalls[0]))

                    first_mm = {"ff": True, "rf": True}
                    for rel, g, Tg, calls in groups:
                        if rel == "ff":
                            sumT_, nwin = sumT_ff, cfg.FWIN
                            tview = ftab[l][0:cfg.SPLIT, :] if g == 0 else ftab[l][cfg.SPLIT:cfg.NF_PAD, :]
                            elem, step = SF, SF
                            dl_t, gp_t = dl_ff, gp[(l, "ff")]
                            idx_t = idx_ff
                            colpt = sumT_ * 8 * w + (0 if g == 0 else relT["ff"][0] * 8)
                            tcol0 = w * sumT_ + (0 if g == 0 else relT["ff"][0])
                            ps = ps_ff
                            Mk = "ff"
                            scol = D + 1
                        else:
                            sumT_, nwin = sumT_rf, cfg.FWIN
                            tview = rtab[l][:, 0:RRO]
                            elem, step = RRO, SR
                            dl_t, gp_t = dl_rf, gp[(l, "rf")]
                            idx_t = idx_rf
                            colpt = sumT_ * 8 * w
                            tcol0 = w * sumT_
                            ps = ps_rf
                            Mk = "rf"
                            scol = D + 1
                        gb = esb.tile([P, Tg * elem], f32, tag=f"gb{rel}{g}")
                        gb3 = gb[:].rearrange("p (t e) -> p t e", e=elem)
                        for off, ntl in calls:
                            ni = ntl * P
                            nc.gpsimd.dma_gather(
                                out_ap=gb3[:, off:off + ntl, :],
                                in_ap=tview,
                                idxs_ap=idx_t[:, colpt:colpt + ni // 16],
                                num_idxs=ni, num_idxs_reg=ni,
                                elem_size=elem, elem_step=step,
                                queue_num=nextq())
                            colpt += ni // 16
                        dexp = scr.tile([P, Tg], f32, tag=f"dx{rel}{g}")
                        for t in range(Tg):
                            s_scr = scr.tile([P, P], f32, tag="sc")
                            nc.vector.scalar_tensor_tensor(
                                out=s_scr[:], in0=iota_t[:],
                                scalar=dl_t[:, tcol0 + t:tcol0 + t + 1],
                                in1=Ms[Mk][:], op0=OP.is_equal, op1=OP.mult,
                                accum_out=dexp[:, t:t + 1])
                        ut = scr.tile([P, Tg], f32, tag=f"u{rel}{g}")
                        nc.vector.tensor_add(ut[:], gp_t[:, tcol0:tcol0 + Tg],
                                             gb3[:, :, scol])
                        nc.vector.tensor_add(ut[:], ut[:], dexp[:])
                        nc.vector.scalar_tensor_tensor(
                            out=ut[:], in0=ut[:], scalar=LEAKY, in1=ut[:],
                            op0=OP.mult, op1=OP.max)
                        nc.scalar.activation(out=ut[:], in_=ut[:], func=AF.Exp)
                        for t in range(Tg):
                            su = scr.tile([P, P], f32, tag="su")
                            nc.vector.scalar_tensor_tensor(
                                out=su[:], in0=iota_t[:],
                                scalar=dl_t[:, tcol0 + t:tcol0 + t + 1],
                                in1=ut[:, t:t + 1].to_broadcast([P, P]),
                                op0=OP.is_equal, op1=OP.mult)
                            last = (rel == "rf" or g == 1) and t == Tg - 1
                            nc.tensor.matmul(ps[:], su[:], gb3[:, t, 0:D + 1],
                                             start=first_mm[rel], stop=last)
                            first_mm[rel] = False
                    # finalize window
                    r_ff = scr.tile([P, 1], f32, tag="rff")
                    nc.vector.tensor_scalar_add(out=r_ff[:], in0=ps_ff[:, D:D + 1], scalar1=EPS)
                    nc.vector.reciprocal(r_ff[:], r_ff[:])
                    r_rf = scr.tile([P, 1], f32, tag="rrf")
                    nc.vector.tensor_scalar_add(out=r_rf[:], in0=ps_rf[:, D:D + 1], scalar1=EPS)
                    nc.vector.reciprocal(r_rf[:], r_rf[:])
                    x2 = esb.tile([P, D], f32, tag="x2")
                    nc.vector.tensor_scalar_mul(out=x2[:], in0=ps_ff[:, 0:D], scalar1=r_ff[:, 0:1])
                    nc.vector.scalar_tensor_tensor(
                        out=x2[:], in0=ps_rf[:, 0:D], scalar=r_rf[:, 0:1],
                        in1=x2[:], op0=OP.mult, op1=OP.add)
                    nc.scalar.activation(out=x2[:], in_=x2[:], func=AF.Relu)
                    if l < 2:
                        nchunk = D // P if D >= P else 1
                        cw = min(D, P)
                        for ch in range(max(1, D // P) if D >= P else 1):
                            tp = mps.tile([P, P], f32, tag="m")
                            nc.tensor.transpose(
                                tp[:cw, :], x2[:, ch * P:ch * P + cw], ident[:])
                            xc = esb.tile([P, P], f32, tag="xc2")
                            nc.vector.tensor_copy(xc[:cw, :], tp[:cw, :])
                            nc.sync.dma_start(
                                xfT[l + 1][ch * P:ch * P + cw, w * P:(w + 1) * P],
                                xc[:cw, :])
                    else:
                        # heads input: transpose to xf3T
                        tp = mps.tile([P, P], f32, tag="m")
                        nc.tensor.transpose(tp[:64, :], x2[:, 0:64], ident[:])
                        xc = esb.tile([P, P], f32, tag="xc2")
                        nc.vector.tensor_copy(xc[:64, :], tp[:64, :])
                        nc.sync.dma_start(
                            xfT[3][0:64, w * P:(w + 1) * P], xc[:64, :])

                # ---- edge phase: room windows (rr); xr unused after layer 2 ----
                idx_rr, dl_rr = st_arr["rr"]["idx"], st_arr["rr"]["dl"]
                for w in range(cfg.RWIN if l < 2 else 0):
                    sc = esb.tile([P, 1], f32, tag="sdrr")
                    nc.sync.dma_start(sc[:], sd_rr[l][w * P:(w + 1) * P, :])
                    mp = mps.tile([P, P], f32, tag="m")
                    nc.tensor.transpose(mp[:], sc[:, 0:1].to_broadcast([P, P]), ident[:])
                    mm = esb.tile([P, P], f32, tag="Mrr")
                    nc.vector.tensor_copy(mm[:], mp[:])
                    ps = eps.tile([P, 15], f32, tag="rr")
                    Tg = TRR
                    gb = esb.tile([P, Tg * 64], f32, tag="gbrr")
                    gb3 = gb[:].rearrange("p (t e) -> p t e", e=64)
                    colpt = sumT_rr * 8 * w
                    tcol0 = w * sumT_rr
                    for off, ntl in rrcalls[0]:
                        ni = ntl * P
                        nc.gpsimd.dma_gather(
                            out_ap=gb3[:, off:off + ntl, :],
                            in_ap=rtab[l][:, RRO:RRO + 64],
                            idxs_ap=idx_rr[:, colpt:colpt + ni // 16],
                            num_idxs=ni, num_idxs_reg=ni,
                            elem_size=64, elem_step=SR,
                            queue_num=nextq())
                        colpt += ni // 16
                    dexp = scr.tile([P, Tg], f32, tag="dxrr")
                    for t in range(Tg):
                        s_scr = scr.tile([P, P], f32, tag="sc")
                        nc.vector.scalar_tensor_tensor(
                            out=s_scr[:], in0=iota_t[:],
                            scalar=dl_rr[:, tcol0 + t:tcol0 + t + 1],
                            in1=mm[:], op0=OP.is_equal, op1=OP.mult,
                            accum_out=dexp[:, t:t + 1])
                    ut = scr.tile([P, Tg], f32, tag="urr")
                    nc.vector.tensor_add(ut[:], gp[(l, "rr")][:, tcol0:tcol0 + Tg],
                                         gb3[:, :, 15])
                    nc.vector.tensor_add(ut[:], ut[:], dexp[:])
                    nc.vector.scalar_tensor_tensor(
                        out=ut[:], in0=ut[:], scalar=LEAKY, in1=ut[:],
                        op0=OP.mult, op1=OP.max)
                    nc.scalar.activation(out=ut[:], in_=ut[:], func=AF.Exp)
                    for t in range(Tg):
                        su = scr.tile([P, P], f32, tag="su")
                        nc.vector.scalar_tensor_tensor(
                            out=su[:], in0=iota_t[:],
                            scalar=dl_rr[:, tcol0 + t:tcol0 + t + 1],
                            in1=ut[:, t:t + 1].to_broadcast([P, P]),
                            op0=OP.is_equal, op1=OP.mult)
                        nc.tensor.matmul(ps[:], su[:], gb3[:, t, 0:15],
                                         start=(t == 0), stop=(t == Tg - 1))
                    r_rr = scr.tile([P, 1], f32, tag="rr1")
                    nc.vector.tensor_scalar_add(out=r_rr[:], in0=ps[:, 14:15], scalar1=EPS)
                    nc.vector.reciprocal(r_rr[:], r_rr[:])
                    x2r = esb.tile([P, 14], f32, tag="x2r")
                    nc.vector.tensor_scalar_mul(out=x2r[:], in0=ps[:, 0:14], scalar1=r_rr[:, 0:1])
                    nc.scalar.activation(out=x2r[:], in_=x2r[:], func=AF.Relu)
                    if l < 2:
                        tp = mps.tile([P, P], f32, tag="m")
                        nc.tensor.transpose(tp[:14, :], x2r[:, 0:14], ident[:])
                        xc = esb.tile([P, P], f32, tag="xc2")
                        nc.vector.tensor_copy(xc[:14, :], tp[:14, :])
                        nc.sync.dma_start(xrT[l + 1][0:14, w * P:(w + 1) * P], xc[:14, :])

        # ---------------- heads ----------------
        with tc.tile_pool(name="hps", bufs=2, space="PSUM") as hps, \
             tc.tile_pool(name="hsb", bufs=3) as hsb:
            bmc = const.tile([P, 1], f32)
            nc.sync.dma_start(bmc[:], inp["bm"][:])
            bvc = const.tile([P, 1], f32)
            nc.sync.dma_start(bvc[:], inp["bv"][:])
            bmat = {}
            for nmk, bc in (("m", bmc), ("v", bvc)):
                mp = hps.tile([P, P], f32, tag="bt")
                nc.tensor.transpose(mp[:], bc[:, 0:1].to_broadcast([P, P]), ident[:])
                bm_t = const.tile([P, P], f32, tag=f"bm{nmk}")
                nc.vector.tensor_copy(bm_t[:], mp[:])
                bmat[nmk] = bm_t
            for m in range(cfg.FWIN):
                a = hsb.tile([64, P], f32, tag="h_in")
                nc.sync.dma_start(a[:], xfT[3][0:64, m * P:(m + 1) * P])
                for nmk, wt, outt in (("m", whm_t, mu_out), ("v", whv_t, lv_out)):
                    ps = hps.tile([P, 64], f32, tag=f"h{nmk}")
                    nc.tensor.matmul(ps[:], a[:], wt[:], start=True, stop=True)
                    ot = hsb.tile([P, 64], f32, tag=f"o{nmk}")
                    nc.vector.tensor_add(ot[:], ps[:], bmat[nmk][:, 0:64])
                    nc.scalar.activation(out=ot[:], in_=ot[:], func=AF.Tanh)
                    nc.sync.dma_start(outt[m * P:(m + 1) * P, :], ot[:])

        _stk.close()
    nc.compile()
    return nc


def _run(inputs, cfg, trace=False):
    in_maps, T, smalls = _host_prep(cfg, inputs)
    shapes = {k: v.shape for k, v in in_maps[0].items()}
    nc = _build(cfg, T, smalls, shapes)
    res = bass_utils.run_bass_kernel_spmd(
        nc, in_maps, core_ids=list(range(NCORES)), trace=trace,
        trace_cores=[0] if trace else None)
    mu = np.concatenate(
        [res.results[c]["mu"][:cfg.nfc_real] for c in range(NCORES)], 0)
    lv = np.concatenate(
        [res.results[c]["lv"][:cfg.nfc_real] for c in range(NCORES)], 0)
    return mu, lv, res.exec_time_ns


def kernel(**inputs):
    cfg = Cfg(50000, 20000)
    mu, lv, _ = _run(inputs, cfg)
    return mu, lv


if __name__ == "__main__":
    pass


# revision 15
# speedup vs baseline: 1.0895x; 1.0895x over previous
"""Trainium2 Bass kernel: 3-layer heterogeneous graph attention encoder.

Sharding: dst-node (edge-cut) partition over 8 NeuronCores. Each core owns a
contiguous range of furniture and room nodes (padded to multiples of 128),
computes the dense per-node projections for its rows, all-gathers the
projection tables, then processes the edges whose destination it owns:
random-row dma_gather of source projections, per-edge softmax weights via
fused one-hot ops, and PSUM-accumulated one-hot matmul aggregation per
128-dst window.
"""

import math
import numpy as np

import concourse.bass as bass
import concourse.bacc as bacc
import concourse.tile as tile
from concourse import mybir
from concourse import bass_utils
from concourse.masks import make_identity

P = 128
NCORES = 8
FURN_DIMS = [(1040, 256), (256, 128), (128, 64)]
ROOM_D = 14
LEAKY = 0.01
NEGBIG = -1.0e30
EPS = 1.0e-9


class Cfg:
    def __init__(self, n_furn, n_room):
        assert n_furn % NCORES == 0 and n_room % NCORES == 0
        self.NF, self.NR = n_furn, n_room
        self.nfc_real = n_furn // NCORES
        self.nrc_real = n_room // NCORES
        self.NFC = ((self.nfc_real + P - 1) // P) * P
        self.NRC = ((self.nrc_real + P - 1) // P) * P
        self.FWIN = self.NFC // P
        self.RWIN = self.NRC // P
        self.NF_PAD = self.NFC * NCORES
        self.NR_PAD = self.NRC * NCORES
        # A/B split for int16 gather indices (furniture tables only)
        self.SPLIT = (self.NF_PAD // 2 + P - 1) // P * P
        assert self.SPLIT < 32768 and self.NF_PAD - self.SPLIT <= 32768
        assert self.NR_PAD < 32768
        # table row strides (fp32 elems, multiples of 64) and rr part offset
        self.SF = []
        self.SR = []
        self.RRO = []
        for (_, d) in FURN_DIMS:
            self.SF.append(((d + 2 + 63) // 64) * 64)
            rro = ((d + 2 + 63) // 64) * 64
            self.RRO.append(rro)
            self.SR.append(rro + 64)

    def furn_pad_id(self, orig):
        return orig + (self.NFC - self.nfc_real) * (orig // self.nfc_real)

    def room_pad_id(self, orig):
        return orig + (self.NRC - self.nrc_real) * (orig // self.nrc_real)


def _prep_rel(cfg, src_new, dst_new, efeat, n_core_pad, nwin, split=None):
    """Per-core padded edge-slot arrays for one relation.

    Returns (T_list, per_core) where per_core[c] is a dict of numpy arrays and
    T_list gives uniform tiles-per-window for each src group.
    """
    c = efeat.shape[1]
    ngrp = 2 if split is not None else 1
    owner = dst_new // n_core_pad
    ldst = dst_new % n_core_pad
    win = ldst // P
    dloc = (ldst % P).astype(np.float32)
    grp = (src_new >= split).astype(np.int64) if split is not None else np.zeros_like(src_new)

    key = (owner * nwin + win) * ngrp + grp
    order = np.argsort(key, kind="stable")
    key_s = key[order]
    src_s = src_new[order]
    dloc_s = dloc[order]
    ef_s = efeat[order]

    nbuck = NCORES * nwin * ngrp
    starts = np.searchsorted(key_s, np.arange(nbuck))
    ends = np.searchsorted(key_s, np.arange(nbuck) + 1)
    counts = (ends - starts).reshape(NCORES, nwin, ngrp)
    T = [int(math.ceil(max(1, counts[:, :, g].max()) / P)) for g in range(ngrp)]
    sumT = sum(T)
    toff = np.concatenate([[0], np.cumsum(T)])[:-1]

    per_core = []
    for co in range(NCORES):
        nslot = nwin * sumT * P
        idx_sl = np.zeros(nslot, np.int64)
        dl_sl = np.zeros(nslot, np.float32)
        pb_sl = np.full(nslot, NEGBIG, np.float32)
        ef_sl = np.zeros((nslot, c), np.float32)
        for w in range(nwin):
            for g in range(ngrp):
                b = (co * nwin + w) * ngrp + g
                s, e = starts[b], ends[b]
                n = e - s
                base = (w * sumT + toff[g]) * P
                if n:
                    sl = slice(base, base + n)
                    ids = src_s[s:e]
                    idx_sl[sl] = ids - (split if g == 1 else 0)
                    dl_sl[sl] = dloc_s[s:e]
                    pb_sl[sl] = 0.0
                    ef_sl[sl] = ef_s[s:e]
        ntiles = nslot // P
        dl_arr = np.ascontiguousarray(dl_sl.reshape(ntiles, P).T)
        pb_arr = np.ascontiguousarray(pb_sl.reshape(ntiles, P).T)
        ef_arr = np.ascontiguousarray(
            ef_sl.reshape(ntiles, P, c).transpose(1, 0, 2).reshape(P, ntiles * c))
        # idx array: per (window, group) call stream chunked to <=1024 idxs
        blocks = []
        for w in range(nwin):
            for g in range(ngrp):
                base = (w * sumT + toff[g]) * P
                callsl = idx_sl[base:base + T[g] * P]
                for off in range(0, T[g] * P, 1024):
                    ni = min(1024, T[g] * P - off)
                    blk = callsl[off:off + ni].reshape(ni // 16, 16).T.astype(np.int16)
                    blocks.append(np.tile(blk, (8, 1)))
        idx_arr = np.ascontiguousarray(np.concatenate(blocks, axis=1))
        per_core.append(dict(idx=idx_arr, dl=dl_arr, pb=pb_arr, ef=ef_arr))
    return T, per_core


def _host_prep(cfg, inputs):
    """All host-side graph/weight prep. Returns (in_maps, T-dict, smalls)."""
    p = inputs["params"]
    ffs = cfg.furn_pad_id(np.asarray(inputs["ff_src"], np.int64))
    ffd = cfg.furn_pad_id(np.asarray(inputs["ff_dst"], np.int64))
    rrs = cfg.room_pad_id(np.asarray(inputs["rr_src"], np.int64))
    rrd = cfg.room_pad_id(np.asarray(inputs["rr_dst"], np.int64))
    rfs = cfg.room_pad_id(np.asarray(inputs["rf_src"], np.int64))
    rfd = cfg.furn_pad_id(np.asarray(inputs["rf_dst"], np.int64))

    T_ff, pc_ff = _prep_rel(cfg, ffs, ffd, np.asarray(inputs["e_ff"], np.float32),
                            cfg.NFC, cfg.FWIN, split=cfg.SPLIT)
    T_rr, pc_rr = _prep_rel(cfg, rrs, rrd, np.asarray(inputs["e_rr"], np.float32),
                            cfg.NRC, cfg.RWIN)
    T_rf, pc_rf = _prep_rel(cfg, rfs, rfd, np.asarray(inputs["e_rf"], np.float32),
                            cfg.NFC, cfg.FWIN)

    # node features: per-core feature-major shards, padded
    xf = np.asarray(inputs["x_furn"], np.float32)
    xr = np.asarray(inputs["x_room"], np.float32)
    xftl, xrtl = [], []
    for co in range(NCORES):
        sh = np.zeros((cfg.NFC, xf.shape[1]), np.float32)
        sh[:cfg.nfc_real] = xf[co * cfg.nfc_real:(co + 1) * cfg.nfc_real]
        xftl.append(np.ascontiguousarray(sh.T))
        shr = np.zeros((cfg.NRC, ROOM_D), np.float32)
        shr[:cfg.nrc_real] = xr[co * cfg.nrc_real:(co + 1) * cfg.nrc_real]
        xrtl.append(np.ascontiguousarray(shr.T))

    # big weights per layer
    wf, wr = [], []
    for l, lp in enumerate(p["layers"]):
        d = FURN_DIMS[l][1]
        ws_ff = np.asarray(lp["ff"]["Ws"], np.float32)
        cols = [ws_ff,
                (ws_ff @ np.asarray(lp["ff"]["a_s"], np.float32))[:, None],
                (np.asarray(lp["ff"]["Wd"], np.float32) @ np.asarray(lp["ff"]["a_d"], np.float32))[:, None],
                (np.asarray(lp["rf"]["Wd"], np.float32) @ np.asarray(lp["rf"]["a_d"], np.float32))[:, None]]
        cols.append(np.zeros((cols[0].shape[0], 1), np.float32))
        wf.append(np.ascontiguousarray(np.concatenate(cols, 1)))  # [fin, d+4]
        ws_rf = np.asarray(lp["rf"]["Ws"], np.float32)
        ws_rr = np.asarray(lp["rr"]["Ws"], np.float32)
        colsr = [ws_rf,
                 (ws_rf @ np.asarray(lp["rf"]["a_s"], np.float32))[:, None],
                 ws_rr,
                 (ws_rr @ np.asarray(lp["rr"]["a_s"], np.float32))[:, None],
                 (np.asarray(lp["rr"]["Wd"], np.float32) @ np.asarray(lp["rr"]["a_d"], np.float32))[:, None]]
        colsr.append(np.zeros((ROOM_D, 1), np.float32))
        wr.append(np.ascontiguousarray(np.concatenate(colsr, 1)))  # [14, d+18]

    whm = np.ascontiguousarray(np.asarray(p["wMean"]["W"], np.float32))
    whv = np.ascontiguousarray(np.asarray(p["wLogVar"]["W"], np.float32))
    bm = np.zeros((P, 1), np.float32)
    bm[:64, 0] = np.asarray(p["wMean"]["b"], np.float32)
    bv = np.zeros((P, 1), np.float32)
    bv[:64, 0] = np.asarray(p["wLogVar"]["b"], np.float32)

    iota = np.ascontiguousarray(
        np.arange(P, dtype=np.float32)[None, :].repeat(P, 0))

    smalls = dict(
        a_e=[[np.asarray(lp[r]["a_e"], np.float32) for r in ("ff", "rr", "rf")]
             for lp in p["layers"]],
        We=[[np.asarray(lp[r]["We"], np.float32) for r in ("ff", "rr", "rf")]
            for lp in p["layers"]],
        be=[[np.asarray(lp[r]["be"], np.float32) for r in ("ff", "rr", "rf")]
            for lp in p["layers"]],
    )

    in_maps = []
    for co in range(NCORES):
        m = dict(
            xft=xftl[co], xrt=xrtl[co],
            whm=whm, whv=whv, bm=bm, bv=bv, iota=iota,
            idx_ff=pc_ff[co]["idx"], dl_ff=pc_ff[co]["dl"],
            pb_ff=pc_ff[co]["pb"], e_ff=pc_ff[co]["ef"],
            idx_rr=pc_rr[co]["idx"], dl_rr=pc_rr[co]["dl"],
            pb_rr=pc_rr[co]["pb"], e_rr=pc_rr[co]["ef"],
            idx_rf=pc_rf[co]["idx"], dl_rf=pc_rf[co]["dl"],
            pb_rf=pc_rf[co]["pb"], e_rf=pc_rf[co]["ef"],
        )
        for l in range(3):
            m[f"wf{l}"] = wf[l]
            m[f"wr{l}"] = wr[l]
        in_maps.append(m)
    return in_maps, dict(ff=T_ff, rr=T_rr, rf=T_rf), smalls


def _gather_calls(T):
    """(tile_offset, ntiles) chunks per group call stream, <=8 tiles each."""
    out = []
    for g, t in enumerate(T):
        chunks = []
        off = 0
        while off < t:
            n = min(8, t - off)
            chunks.append((off, n))
            off += n
        out.append(chunks)
    return out


def _build(cfg, T, smalls, in0_shapes):
    f32 = mybir.dt.float32
    f32r = mybir.dt.float32r
    i16 = mybir.dt.int16
    AF = mybir.ActivationFunctionType
    OP = mybir.AluOpType
    nc = bacc.Bacc("TRN2", target_bir_lowering=False, num_devices=NCORES,
                   num_swdge_queues=4)
    qctr = [0]

    def nextq():
        qctr[0] = (qctr[0] + 1) % 4
        return qctr[0]

    # ---------------- I/O -----------------
    inp = {}
    f32r_inputs = {"xft", "xrt", "whm", "whv",
                   "wf0", "wf1", "wf2", "wr0", "wr1", "wr2"}
    for name, shp in in0_shapes.items():
        dt = i16 if name.startswith("idx") else (f32r if name in f32r_inputs else f32)
        inp[name] = nc.dram_tensor(name, list(shp), dt, kind="ExternalInput")
    mu_out = nc.dram_tensor("mu", [cfg.NFC, 64], f32, kind="ExternalOutput")
    lv_out = nc.dram_tensor("lv", [cfg.NFC, 64], f32, kind="ExternalOutput")

    relT = {"ff": T["ff"], "rr": T["rr"], "rf": T["rf"]}
    sumT_ff = sum(relT["ff"])
    sumT_rr = sum(relT["rr"])
    sumT_rf = sum(relT["rf"])
    rg = [list(range(NCORES))]

    from contextlib import ExitStack
    _stk = ExitStack()
    with tile.TileContext(nc) as tc:
        dram = _stk.enter_context(tc.tile_pool(name="dram", bufs=1, space="DRAM"))
        const = _stk.enter_context(tc.tile_pool(name="const", bufs=1))

        # DRAM scratch
        ftab_sh = [dram.tile([cfg.NFC, cfg.SF[l]], f32r, tag=f"fts{l}", name=f"fts{l}") for l in range(3)]
        ftab = [dram.tile([cfg.NF_PAD, cfg.SF[l]], f32r, tag=f"ftf{l}", name=f"ftf{l}", addr_space="Shared") for l in range(3)]
        rtab_sh = [dram.tile([cfg.NRC, cfg.SR[l]], f32r, tag=f"rts{l}", name=f"rts{l}") for l in range(3)]
        rtab = [dram.tile([cfg.NR_PAD, cfg.SR[l]], f32r, tag=f"rtf{l}", name=f"rtf{l}", addr_space="Shared") for l in range(3)]
        sd_ff = [dram.tile([cfg.NFC, 1], f32r, tag=f"sdf{l}", name=f"sdf{l}") for l in range(3)]
        sd_rf = [dram.tile([cfg.NFC, 1], f32r, tag=f"sdr{l}", name=f"sdr{l}") for l in range(3)]
        sd_rr = [dram.tile([cfg.NRC, 1], f32r, tag=f"sdq{l}", name=f"sdq{l}") for l in range(3)]
        xfT = [None,
               dram.tile([FURN_DIMS[0][1], cfg.NFC], f32r, tag="xf1T", name="xf1T"),
               dram.tile([FURN_DIMS[1][1], cfg.NFC], f32r, tag="xf2T", name="xf2T"),
               dram.tile([FURN_DIMS[2][1], cfg.NFC], f32r, tag="xf3T", name="xf3T")]
        xrT = [None, dram.tile([14, cfg.NRC], f32r, tag="xr1T", name="xr1T"),
               dram.tile([14, cfg.NRC], f32r, tag="xr2T", name="xr2T")]

        # constants
        iota_t = const.tile([P, P], f32)
        nc.sync.dma_start(iota_t[:], inp["iota"][:])
        ident = const.tile([P, P], f32)
        make_identity(nc, ident[:])

        # weights to SBUF
        wf_t = []   # per layer: list of [128, cols] chunks
        wr_t = []
        for l in range(3):
            fin, d = FURN_DIMS[l]
            nk = (fin + P - 1) // P
            cols = d + 4
            chunks = []
            for k in range(nk):
                kn = min(P, fin - k * P)
                t_ = const.tile([P, cols], f32r, tag=f"wf{l}_{k}")
                nc.sync.dma_start(t_[:kn, :], inp[f"wf{l}"][k * P:k * P + kn, :])
                chunks.append((t_, kn))
            wf_t.append(chunks)
            t_ = const.tile([ROOM_D, d + 18], f32r, tag=f"wr{l}")
            nc.sync.dma_start(t_[:], inp[f"wr{l}"][:])
            wr_t.append(t_)
        whm_t = const.tile([64, 64], f32r)
        nc.sync.dma_start(whm_t[:], inp["whm"][:])
        whv_t = const.tile([64, 64], f32r)
        nc.sync.dma_start(whv_t[:], inp["whv"][:])

        # per-edge static arrays
        st_arr = {}
        for r, sumT_, c in (("ff", sumT_ff, 3), ("rr", sumT_rr, 4), ("rf", sumT_rf, 5)):
            nwin = cfg.FWIN if r in ("ff", "rf") else cfg.RWIN
            tt = nwin * sumT_
            idx_t = const.tile([P, inp[f"idx_{r}"].shape[1]], i16, tag=f"idx{r}")
            nc.sync.dma_start(idx_t[:], inp[f"idx_{r}"][:])
            dl_t = const.tile([P, tt], f32, tag=f"dl{r}")
            nc.sync.dma_start(dl_t[:], inp[f"dl_{r}"][:])
            st_arr[r] = dict(idx=idx_t, dl=dl_t, tt=tt, c=c, nwin=nwin)

        # ---------------- edge-feature prelude: g_pb per layer/relation ----
        gp = {}  # (l, r) -> [P, tt] tile
        with tc.tile_pool(name="effp", bufs=1) as effp:
            eff = {}
            for r in ("ff", "rr", "rf"):
                c = st_arr[r]["c"]
                tt = st_arr[r]["tt"]
                e0 = effp.tile([P, tt * c], f32, tag=f"e0{r}")
                nc.sync.dma_start(e0[:], inp[f"e_{r}"][:])
                pb = effp.tile([P, tt], f32, tag=f"pb{r}")
                nc.sync.dma_start(pb[:], inp[f"pb_{r}"][:])
                eff[r] = (e0, pb)
            ridx = {"ff": 0, "rr": 1, "rf": 2}
            for l in range(3):
                for r in ("ff", "rr", "rf"):
                    c = st_arr[r]["tt"], st_arr[r]["c"]
                    tt, cc = c
                    e_t, pb_t = eff[r]
                    ae = smalls["a_e"][l][ridx[r]]
                    g_t = const.tile([P, tt], f32, tag=f"gp{l}{r}")
                    ev = e_t[:].rearrange("p (t c) -> p t c", c=cc)
                    nc.vector.scalar_tensor_tensor(
                        out=g_t[:], in0=ev[:, :, 0], scalar=float(ae[0]),
                        in1=pb_t[:], op0=OP.mult, op1=OP.add)
                    for j in range(1, cc):
                        nc.vector.scalar_tensor_tensor(
                            out=g_t[:], in0=ev[:, :, j], scalar=float(ae[j]),
                            in1=g_t[:], op0=OP.mult, op1=OP.add)
                    gp[(l, r)] = g_t
                if l < 2:
                    for r in ("ff", "rr", "rf"):
                        tt, cc = st_arr[r]["tt"], st_arr[r]["c"]
                        e_t, pb_t = eff[r]
                        We = smalls["We"][l][ridx[r]]
                        be = smalls["be"][l][ridx[r]]
                        e_n = effp.tile([P, tt * cc], f32, tag=f"e{l + 1}{r}")
                        ev = e_t[:].rearrange("p (t c) -> p t c", c=cc)
                        en = e_n[:].rearrange("p (t c) -> p t c", c=cc)
                        for j2 in range(cc):
                            nc.vector.tensor_scalar(
                                out=en[:, :, j2], in0=ev[:, :, 0],
                                scalar1=float(We[0, j2]), scalar2=float(be[j2]),
                                op0=OP.mult, op1=OP.add)
                            for j in range(1, cc):
                                nc.vector.scalar_tensor_tensor(
                                    out=en[:, :, j2], in0=ev[:, :, j],
                                    scalar=float(We[j, j2]), in1=en[:, :, j2],
                                    op0=OP.mult, op1=OP.add)
                            nc.vector.tensor_scalar_max(
                                out=en[:, :, j2], in0=en[:, :, j2], scalar1=0.0)
                        eff[r] = (e_n, pb_t)

        # ---------------- per-layer phases ----------------
        for l in range(3):
            fin, D = FURN_DIMS[l]
            SF, SR, RRO = cfg.SF[l], cfg.SR[l], cfg.RRO[l]
            nk = (fin + P - 1) // P

            # ---- node phase: furniture ----
            with tc.tile_pool(name=f"nps{l}", bufs=2, space="PSUM") as nps, \
                 tc.tile_pool(name=f"nsb{l}", bufs=3) as nsb:
                for m in range(cfg.FWIN):
                    ps = nps.tile([P, D + 4], f32, tag="f")
                    for k in range(nk):
                        wt, kn = wf_t[l][k]
                        a = nsb.tile([P, P], f32r, tag="xc")
                        if l == 0:
                            src = inp["xft"]
                        else:
                            src = xfT[l]
                        nc.sync.dma_start(
                            a[:kn, :], src[k * P:k * P + kn, m * P:(m + 1) * P])
                        nc.tensor.matmul(ps[:], a[:kn, :], wt[:kn, :],
                                         start=(k == 0), stop=(k == nk - 1))
                    st = nsb.tile([P, SF], f32r, tag="stg")
                    nc.vector.tensor_copy(st[:, 0:D], ps[:, 0:D])
                    nc.vector.memset(st[:, D:D + 1].bitcast(f32), 1.0)
                    nc.vector.tensor_copy(st[:, D + 1:D + 4], ps[:, D:D + 3])
                    nc.sync.dma_start(ftab_sh[l][m * P:(m + 1) * P, :], st[:])
                    nc.sync.dma_start(sd_ff[l][m * P:(m + 1) * P, :], st[:, D + 2:D + 3])
                    nc.sync.dma_start(sd_rf[l][m * P:(m + 1) * P, :], st[:, D + 3:D + 4])
                # rooms
                for m in range(cfg.RWIN):
                    ps = nps.tile([P, D + 18], f32, tag="r")
                    wt = wr_t[l]
                    a = nsb.tile([ROOM_D, P], f32r, tag="xr")
                    if l == 0:
                        nc.sync.dma_start(a[:], inp["xrt"][:, m * P:(m + 1) * P])
                    else:
                        nc.sync.dma_start(a[:], xrT[l][:, m * P:(m + 1) * P])
                    nc.tensor.matmul(ps[:], a[:], wt[:], start=True, stop=True)
                    st = nsb.tile([P, SR], f32r, tag="stgr")
                    nc.vector.tensor_copy(st[:, 0:D], ps[:, 0:D])
                    nc.vector.memset(st[:, D:D + 1].bitcast(f32), 1.0)
                    nc.vector.tensor_copy(st[:, D + 1:D + 2], ps[:, D:D + 1])
                    nc.vector.tensor_copy(st[:, RRO:RRO + 14], ps[:, D + 1:D + 15])
                    nc.vector.memset(st[:, RRO + 14:RRO + 15].bitcast(f32), 1.0)
                    nc.vector.tensor_copy(st[:, RRO + 15:RRO + 17], ps[:, D + 15:D + 17])
                    nc.sync.dma_start(rtab_sh[l][m * P:(m + 1) * P, :], st[:])
                    nc.sync.dma_start(sd_rr[l][m * P:(m + 1) * P, :], st[:, RRO + 16:RRO + 17])

            nc.gpsimd.collective_compute(
                "AllGather", OP.bypass, ins=[ftab_sh[l].opt()],
                outs=[ftab[l].opt()], replica_groups=rg)
            nc.gpsimd.collective_compute(
                "AllGather", OP.bypass, ins=[rtab_sh[l].opt()],
                outs=[rtab[l].opt()], replica_groups=rg)

            # ---- edge phase: furniture windows (ff + rf) ----
            TA, TB = relT["ff"]
            TRF = relT["rf"][0]
            TRR = relT["rr"][0]
            with tc.tile_pool(name=f"eps{l}", bufs=2, space="PSUM") as eps, \
                 tc.tile_pool(name=f"mps{l}", bufs=2, space="PSUM") as mps, \
                 tc.tile_pool(name=f"esb{l}", bufs=3) as esb, \
                 tc.tile_pool(name=f"scr{l}", bufs=6) as scr:
                idx_ff, dl_ff = st_arr["ff"]["idx"], st_arr["ff"]["dl"]
                idx_rf, dl_rf = st_arr["rf"]["idx"], st_arr["rf"]["dl"]
                ffcalls = _gather_calls(relT["ff"])
                rfcalls = _gather_calls(relT["rf"])
                rrcalls = _gather_calls(relT["rr"])
                for w in range(cfg.FWIN):
                    # M matrices
                    Ms = {}
                    for rel, sdt in (("ff", sd_ff[l]), ("rf", sd_rf[l])):
                        sc = esb.tile([P, 1], f32r, tag=f"sd{rel}")
                        nc.sync.dma_start(sc[:], sdt[w * P:(w + 1) * P, :])
                        mp = mps.tile([P, P], f32, tag="m")
                        nc.tensor.transpose(mp[:], sc[:, 0:1].bitcast(f32).to_broadcast([P, P]), ident[:])
                        mm = esb.tile([P, P], f32, tag=f"M{rel}")
                        nc.vector.tensor_copy(mm[:], mp[:])
                        Ms[rel] = mm
                    ps_ff = eps.tile([P, D + 2], f32, tag="ff")
                    ps_rf = eps.tile([P, D + 2], f32, tag="rf")

                    groups = []
                    for g in range(2):
                        groups.append(("ff", g, relT["ff"][g], ffcalls[g]))
                    groups.append(("rf", 0, TRF, rfcalls[0]))

                    first_mm = {"ff": True, "rf": True}
                    for rel, g, Tg, calls in groups:
                        if rel == "ff":
                            sumT_, nwin = sumT_ff, cfg.FWIN
                            tview = ftab[l][0:cfg.SPLIT, :] if g == 0 else ftab[l][cfg.SPLIT:cfg.NF_PAD, :]
                            elem, step = SF, SF
                            dl_t, gp_t = dl_ff, gp[(l, "ff")]
                            idx_t = idx_ff
                            colpt = sumT_ * 8 * w + (0 if g == 0 else relT["ff"][0] * 8)
                            tcol0 = w * sumT_ + (0 if g == 0 else relT["ff"][0])
                            ps = ps_ff
                            Mk = "ff"
                            scol = D + 1
                        else:
                            sumT_, nwin = sumT_rf, cfg.FWIN
                            tview = rtab[l][:, 0:RRO]
                            elem, step = RRO, SR
                            dl_t, gp_t = dl_rf, gp[(l, "rf")]
                            idx_t = idx_rf
                            colpt = sumT_ * 8 * w
                            tcol0 = w * sumT_
                            ps = ps_rf
                            Mk = "rf"
                            scol = D + 1
                        gb = esb.tile([P, Tg * elem], f32r, tag=f"gb{rel}{g}")
                        gb3 = gb[:].rearrange("p (t e) -> p t e", e=elem)
                        for off, ntl in calls:
                            ni = ntl * P
                            nc.gpsimd.dma_gather(
                                out_ap=gb3[:, off:off + ntl, :],
                                in_ap=tview,
                                idxs_ap=idx_t[:, colpt:colpt + ni // 16],
                                num_idxs=ni, num_idxs_reg=ni,
                                elem_size=elem, elem_step=step,
                                queue_num=nextq())
                            colpt += ni // 16
                        dexp = scr.tile([P, Tg], f32, tag=f"dx{rel}{g}")
                        for t in range(Tg):
                            s_scr = scr.tile([P, P], f32, tag="sc")
                            nc.vector.scalar_tensor_tensor(
                                out=s_scr[:], in0=iota_t[:],
                                scalar=dl_t[:, tcol0 + t:tcol0 + t + 1],
                                in1=Ms[Mk][:], op0=OP.is_equal, op1=OP.mult,
                                accum_out=dexp[:, t:t + 1])
                        ut = scr.tile([P, Tg], f32, tag=f"u{rel}{g}")
                        nc.vector.tensor_add(ut[:], gp_t[:, tcol0:tcol0 + Tg],
                                             gb3[:, :, scol].bitcast(f32))
                        nc.vector.tensor_add(ut[:], ut[:], dexp[:])
                        nc.vector.scalar_tensor_tensor(
                            out=ut[:], in0=ut[:], scalar=LEAKY, in1=ut[:],
                            op0=OP.mult, op1=OP.max)
                        nc.scalar.activation(out=ut[:], in_=ut[:], func=AF.Exp)
                        for t in range(Tg):
                            su = scr.tile([P, P], f32r, tag="su")
                            nc.vector.scalar_tensor_tensor(
                                out=su[:], in0=iota_t[:],
                                scalar=dl_t[:, tcol0 + t:tcol0 + t + 1],
                                in1=ut[:, t:t + 1].to_broadcast([P, P]),
                                op0=OP.is_equal, op1=OP.mult)
                            last = (rel == "rf" or g == 1) and t == Tg - 1
                            nc.tensor.matmul(ps[:], su[:], gb3[:, t, 0:D + 2],
                                             start=first_mm[rel], stop=last)
                            first_mm[rel] = False
                    # finalize window
                    r_ff = scr.tile([P, 1], f32, tag="rff")
                    nc.vector.tensor_scalar_add(out=r_ff[:], in0=ps_ff[:, D:D + 1], scalar1=EPS)
                    nc.vector.reciprocal(r_ff[:], r_ff[:])
                    r_rf = scr.tile([P, 1], f32, tag="rrf")
                    nc.vector.tensor_scalar_add(out=r_rf[:], in0=ps_rf[:, D:D + 1], scalar1=EPS)
                    nc.vector.reciprocal(r_rf[:], r_rf[:])
                    x2 = esb.tile([P, D], f32, tag="x2")
                    nc.vector.tensor_scalar_mul(out=x2[:], in0=ps_ff[:, 0:D], scalar1=r_ff[:, 0:1])
                    nc.vector.scalar_tensor_tensor(
                        out=x2[:], in0=ps_rf[:, 0:D], scalar=r_rf[:, 0:1],
                        in1=x2[:], op0=OP.mult, op1=OP.add)
                    nc.scalar.activation(out=x2[:], in_=x2[:], func=AF.Relu)
                    if l < 2:
                        nchunk = D // P if D >= P else 1
                        cw = min(D, P)
                        for ch in range(max(1, D // P) if D >= P else 1):
                            tp = mps.tile([P, P], f32, tag="m")
                            nc.tensor.transpose(
                                tp[:cw, :], x2[:, ch * P:ch * P + cw], ident[:])
                            xc = esb.tile([P, P], f32r, tag="xc2")
                            nc.vector.tensor_copy(xc[:cw, :], tp[:cw, :])
                            nc.sync.dma_start(
                                xfT[l + 1][ch * P:ch * P + cw, w * P:(w + 1) * P],
                                xc[:cw, :])
                    else:
                        # heads input: transpose to xf3T
                        tp = mps.tile([P, P], f32, tag="m")
                        nc.tensor.transpose(tp[:64, :], x2[:, 0:64], ident[:])
                        xc = esb.tile([P, P], f32r, tag="xc2")
                        nc.vector.tensor_copy(xc[:64, :], tp[:64, :])
                        nc.sync.dma_start(
                            xfT[3][0:64, w * P:(w + 1) * P], xc[:64, :])

                # ---- edge phase: room windows (rr); xr unused after layer 2 ----
                idx_rr, dl_rr = st_arr["rr"]["idx"], st_arr["rr"]["dl"]
                for w in range(cfg.RWIN if l < 2 else 0):
                    sc = esb.tile([P, 1], f32r, tag="sdrr")
                    nc.sync.dma_start(sc[:], sd_rr[l][w * P:(w + 1) * P, :])
                    mp = mps.tile([P, P], f32, tag="m")
                    nc.tensor.transpose(mp[:], sc[:, 0:1].bitcast(f32).to_broadcast([P, P]), ident[:])
                    mm = esb.tile([P, P], f32, tag="Mrr")
                    nc.vector.tensor_copy(mm[:], mp[:])
                    ps = eps.tile([P, 16], f32, tag="rr")
                    Tg = TRR
                    gb = esb.tile([P, Tg * 64], f32r, tag="gbrr")
                    gb3 = gb[:].rearrange("p (t e) -> p t e", e=64)
                    colpt = sumT_rr * 8 * w
                    tcol0 = w * sumT_rr
                    for off, ntl in rrcalls[0]:
                        ni = ntl * P
                        nc.gpsimd.dma_gather(
                            out_ap=gb3[:, off:off + ntl, :],
                            in_ap=rtab[l][:, RRO:RRO + 64],
                            idxs_ap=idx_rr[:, colpt:colpt + ni // 16],
                            num_idxs=ni, num_idxs_reg=ni,
                            elem_size=64, elem_step=SR,
                            queue_num=nextq())
                        colpt += ni // 16
                    dexp = scr.tile([P, Tg], f32, tag="dxrr")
                    for t in range(Tg):
                        s_scr = scr.tile([P, P], f32, tag="sc")
                        nc.vector.scalar_tensor_tensor(
                            out=s_scr[:], in0=iota_t[:],
                            scalar=dl_rr[:, tcol0 + t:tcol0 + t + 1],
                            in1=mm[:], op0=OP.is_equal, op1=OP.mult,
                            accum_out=dexp[:, t:t + 1])
                    ut = scr.tile([P, Tg], f32, tag="urr")
                    nc.vector.tensor_add(ut[:], gp[(l, "rr")][:, tcol0:tcol0 + Tg],
                                         gb3[:, :, 15].bitcast(f32))
                    nc.vector.tensor_add(ut[:], ut[:], dexp[:])
                    nc.vector.scalar_tensor_tensor(
                        out=ut[:], in0=ut[:], scalar=LEAKY, in1=ut[:],
                        op0=OP.mult, op1=OP.max)
                    nc.scalar.activation(out=ut[:], in_=ut[:], func=AF.Exp)
                    for t in range(Tg):
                        su = scr.tile([P, P], f32r, tag="su")
                        nc.vector.scalar_tensor_tensor(
                            out=su[:], in0=iota_t[:],
                            scalar=dl_rr[:, tcol0 + t:tcol0 + t + 1],
                            in1=ut[:, t:t + 1].to_broadcast([P, P]),
                            op0=OP.is_equal, op1=OP.mult)
                        nc.tensor.matmul(ps[:], su[:], gb3[:, t, 0:16],
                                         start=(t == 0), stop=(t == Tg - 1))
                    r_rr = scr.tile([P, 1], f32, tag="rr1")
                    nc.vector.tensor_scalar_add(out=r_rr[:], in0=ps[:, 14:15], scalar1=EPS)
                    nc.vector.reciprocal(r_rr[:], r_rr[:])
                    x2r = esb.tile([P, 14], f32, tag="x2r")
                    nc.vector.tensor_scalar_mul(out=x2r[:], in0=ps[:, 0:14], scalar1=r_rr[:, 0:1])
                    nc.scalar.activation(out=x2r[:], in_=x2r[:], func=AF.Relu)
                    if l < 2:
                        tp = mps.tile([P, P], f32, tag="m")
                        nc.tensor.transpose(tp[:14, :], x2r[:, 0:14], ident[:])
                        xc = esb.tile([P, P], f32r, tag="xc2")
                        nc.vector.tensor_copy(xc[:14, :], tp[:14, :])
                        nc.sync.dma_start(xrT[l + 1][0:14, w * P:(w + 1) * P], xc[:14, :])

        # ---------------- heads ----------------
        with tc.tile_pool(name="hps", bufs=2, space="PSUM") as hps, \
             tc.tile_pool(name="hsb", bufs=3) as hsb:
            bmc = const.tile([P, 1], f32)
            nc.sync.dma_start(bmc[:], inp["bm"][:])
            bvc = const.tile([P, 1], f32)
            nc.sync.dma_start(bvc[:], inp["bv"][:])
            bmat = {}
            for nmk, bc in (("m", bmc), ("v", bvc)):
                mp = hps.tile([P, P], f32, tag="bt")
                nc.tensor.transpose(mp[:], bc[:, 0:1].to_broadcast([P, P]), ident[:])
                bm_t = const.tile([P, P], f32, tag=f"bm{nmk}")
                nc.vector.tensor_copy(bm_t[:], mp[:])
                bmat[nmk] = bm_t
            for m in range(cfg.FWIN):
                a = hsb.tile([64, P], f32r, tag="h_in")
                nc.sync.dma_start(a[:], xfT[3][0:64, m * P:(m + 1) * P])
                for nmk, wt, outt in (("m", whm_t, mu_out), ("v", whv_t, lv_out)):
                    ps = hps.tile([P, 64], f32, tag=f"h{nmk}")
                    nc.tensor.matmul(ps[:], a[:], wt[:], start=True, stop=True)
                    ot = hsb.tile([P, 64], f32, tag=f"o{nmk}")
                    nc.vector.tensor_add(ot[:], ps[:], bmat[nmk][:, 0:64])
                    nc.scalar.activation(out=ot[:], in_=ot[:], func=AF.Tanh)
                    nc.sync.dma_start(outt[m * P:(m + 1) * P, :], ot[:])

        _stk.close()
    nc.compile()
    return nc


def _run(inputs, cfg, trace=False):
    in_maps, T, smalls = _host_prep(cfg, inputs)
    shapes = {k: v.shape for k, v in in_maps[0].items()}
    nc = _build(cfg, T, smalls, shapes)
    res = bass_utils.run_bass_kernel_spmd(
        nc, in_maps, core_ids=list(range(NCORES)), trace=trace,
        trace_cores=[0] if trace else None)
    mu = np.concatenate(
        [res.results[c]["mu"][:cfg.nfc_real] for c in range(NCORES)], 0)
    lv = np.concatenate(
        [res.results[c]["lv"][:cfg.nfc_real] for c in range(NCORES)], 0)
    return mu, lv, res.exec_time_ns


def kernel(**inputs):
    cfg = Cfg(50000, 20000)
    mu, lv, _ = _run(inputs, cfg)
    return mu, lv


if __name__ == "__main__":
    pass
